# revision 3
# baseline (speedup 1.0000x reference)
"""Trainium2 Bass kernel: gather-rate-scatter metabolite update (one Euler
kinetics step) for B=262144 cells, data-parallel across 8 NeuronCores.

Math (per cell batch):
    enzyme = gene_expr @ G                      [B, 64]
    rates  = kcat * sigmoid(enzyme) * exp(log(conc+eps) @ max(-S,0))
    out    = max(conc + DT * (rates @ S.T), 0)  [B, 114]

Device strategy: shard B across 8 cores (pure data parallelism). On the host
the per-core shards are transposed so features sit on SBUF partitions; the
three small contractions (K=212 split 128+84, K=114, K=64) then run natively
on the tensor engine with zero on-chip transposes and fully contiguous DMA.
The gene path uses fp16 (G is exactly representable; halves its HBM traffic);
the conc/log path stays fp32. kcat folds into the Exp activation bias
(exp(x + ln kcat)), DT folds into the scatter weights (DT*S).
"""

import threading

import numpy as np

N_METS = 114
N_RXNS = 64
N_GENES = 212
B = 262144
N_CORES = 8
BS = B // N_CORES      # 32768 cells per core
CT = 4096              # cells per SBUF tile (DMA granularity)
NC = 512               # cells per PSUM chunk (one fp32 bank)
G_HI = 128             # gene split: partitions 0..127
G_LO = N_GENES - G_HI  # remaining 84 gene rows
DT = 0.01
EPS = 1e-6

_lock = threading.Lock()
_cached = {}


def _build_program():
    import concourse.mybir as mybir
    import concourse.tile as tile
    from concourse import bacc

    f32 = mybir.dt.float32
    f16 = mybir.dt.float16
    AF = mybir.ActivationFunctionType

    nc = bacc.Bacc(
        "TRN2", target_bir_lowering=False, debug=False, num_devices=N_CORES
    )
    d_conc = nc.dram_tensor("conc_t", [N_METS, BS], f32, kind="ExternalInput").ap()
    d_ghi = nc.dram_tensor("gene_hi", [G_HI, BS], f16, kind="ExternalInput").ap()
    d_glo = nc.dram_tensor("gene_lo", [G_LO, BS], f16, kind="ExternalInput").ap()
    d_Ghi = nc.dram_tensor("g_map_hi", [G_HI, N_RXNS], f16, kind="ExternalInput").ap()
    d_Glo = nc.dram_tensor("g_map_lo", [G_LO, N_RXNS], f16, kind="ExternalInput").ap()
    d_sneg = nc.dram_tensor("s_neg", [N_METS, N_RXNS], f32, kind="ExternalInput").ap()
    d_sdt = nc.dram_tensor("s_dt", [N_RXNS, N_METS], f16, kind="ExternalInput").ap()
    d_lnk = nc.dram_tensor("ln_kcat", [N_RXNS, 1], f32, kind="ExternalInput").ap()
    d_out = nc.dram_tensor("out_t", [N_METS, BS], f32, kind="ExternalOutput").ap()

    with tile.TileContext(nc) as tc:
        with (
            tc.tile_pool(name="consts", bufs=1) as consts,
            tc.tile_pool(name="io", bufs=2) as io,
            tc.tile_pool(name="mid", bufs=3) as mid,
            tc.tile_pool(name="ps_e", bufs=2, space="PSUM") as ps_e,
            tc.tile_pool(name="ps_p", bufs=2, space="PSUM") as ps_p,
            tc.tile_pool(name="ps_d", bufs=2, space="PSUM") as ps_d,
        ):
            c_Ghi = consts.tile([G_HI, N_RXNS], f16)
            nc.sync.dma_start(out=c_Ghi, in_=d_Ghi)
            c_Glo = consts.tile([G_LO, N_RXNS], f16)
            nc.sync.dma_start(out=c_Glo, in_=d_Glo)
            c_sneg = consts.tile([N_METS, N_RXNS], f32)
            nc.sync.dma_start(out=c_sneg, in_=d_sneg)
            c_sdt = consts.tile([N_RXNS, N_METS], f16)
            nc.sync.dma_start(out=c_sdt, in_=d_sdt)
            c_lnk = consts.tile([N_RXNS, 1], f32)
            nc.sync.dma_start(out=c_lnk, in_=d_lnk)
            c_eps = consts.tile([N_METS, 1], f32)
            nc.vector.memset(c_eps, EPS)

            for it in range(BS // CT):
                sl = slice(it * CT, (it + 1) * CT)
                t_conc = io.tile([N_METS, CT], f32, tag="conc")
                nc.sync.dma_start(out=t_conc, in_=d_conc[:, sl])
                t_ghi = io.tile([G_HI, CT], f16, tag="ghi")
                nc.sync.dma_start(out=t_ghi, in_=d_ghi[:, sl])
                t_glo = io.tile([G_LO, CT], f16, tag="glo")
                nc.sync.dma_start(out=t_glo, in_=d_glo[:, sl])
                t_out = io.tile([N_METS, CT], f32, tag="out")

                for ic in range(CT // NC):
                    cs = slice(ic * NC, (ic + 1) * NC)
                    # enzyme = gene @ G  (contract genes: 128 + 84)
                    p_e = ps_e.tile([N_RXNS, NC], f32, tag="pe")
                    nc.tensor.matmul(p_e, c_Ghi, t_ghi[:, cs], start=True, stop=False)
                    nc.tensor.matmul(p_e, c_Glo, t_glo[:, cs], start=False, stop=True)
                    # log-space substrate gather
                    t_logc = mid.tile([N_METS, NC], f32, tag="logc")
                    nc.scalar.activation(t_logc, t_conc[:, cs], AF.Ln, bias=c_eps)
                    p_p = ps_p.tile([N_RXNS, NC], f32, tag="pp")
                    nc.tensor.matmul(p_p, c_sneg, t_logc, start=True, stop=True)
                    # rates = sigmoid(enzyme) * exp(log-prod + ln kcat)
                    t_sig = mid.tile([N_RXNS, NC], f16, tag="sig")
                    nc.scalar.activation(t_sig, p_e, AF.Sigmoid)
                    t_ex = mid.tile([N_RXNS, NC], f16, tag="ex")
                    nc.scalar.activation(t_ex, p_p, AF.Exp, bias=c_lnk)
                    t_rates = mid.tile([N_RXNS, NC], f16, tag="rates")
                    nc.vector.tensor_mul(t_rates, t_sig, t_ex)
                    # scatter: DT * (rates @ S.T), DT pre-folded into s_dt
                    p_d = ps_d.tile([N_METS, NC], f32, tag="pd")
                    nc.tensor.matmul(p_d, c_sdt, t_rates, start=True, stop=True)
                    # out = relu(conc + DT*d_conc)
                    nc.vector.tensor_add(t_out[:, cs], t_conc[:, cs], p_d)
                    nc.vector.tensor_scalar_max(t_out[:, cs], t_out[:, cs], 0.0)

                nc.sync.dma_start(out=d_out[:, sl], in_=t_out)

    nc.compile()
    return nc


def _get_program():
    with _lock:
        if "nc" not in _cached:
            _cached["nc"] = _build_program()
        return _cached["nc"]


def kernel(conc, gene_expr, S, G, kcat):
    from concourse.bass_utils import run_bass_kernel_spmd

    conc = np.asarray(conc, dtype=np.float32)
    gene_expr = np.asarray(gene_expr, dtype=np.float32)
    S = np.asarray(S, dtype=np.float32)
    G = np.asarray(G, dtype=np.float32)
    kcat = np.asarray(kcat, dtype=np.float32)

    nc = _get_program()

    g_f16 = G.astype(np.float16)
    consts = {
        "g_map_hi": np.ascontiguousarray(g_f16[:G_HI]),
        "g_map_lo": np.ascontiguousarray(g_f16[G_HI:]),
        "s_neg": np.ascontiguousarray(np.maximum(-S, 0.0).astype(np.float32)),
        "s_dt": np.ascontiguousarray((DT * S).T.astype(np.float16)),
        "ln_kcat": np.log(kcat).astype(np.float32).reshape(N_RXNS, 1),
    }

    in_maps = []
    for c in range(N_CORES):
        rows = slice(c * BS, (c + 1) * BS)
        gene_t = gene_expr[rows].T.astype(np.float16)
        in_maps.append(
            {
                "conc_t": np.ascontiguousarray(conc[rows].T),
                "gene_hi": np.ascontiguousarray(gene_t[:G_HI]),
                "gene_lo": np.ascontiguousarray(gene_t[G_HI:]),
                **consts,
            }
        )

    res = run_bass_kernel_spmd(nc, in_maps, core_ids=list(range(N_CORES)))

    out = np.empty((B, N_METS), dtype=np.float32)
    for c in range(N_CORES):
        out[c * BS : (c + 1) * BS] = res.results[c]["out_t"].T
    return out


# revision 4
# speedup vs baseline: 2.2390x; 2.2390x over previous
"""Trainium2 Bass kernel: gather-rate-scatter metabolite update (one Euler
kinetics step) for B=262144 cells, data-parallel across 8 NeuronCores.

Math (per cell batch):
    enzyme = gene_expr @ G                      [B, 64]
    rates  = kcat * sigmoid(enzyme) * exp(log(conc+eps) @ max(-S,0))
    out    = max(conc + DT * (rates @ S.T), 0)  [B, 114]

Device strategy: shard B across 8 cores (pure data parallelism). On the host
the per-core shards are transposed so features sit on SBUF partitions; the
small contractions then run natively on the tensor engine with zero on-chip
transposes and fully contiguous DMA.

Key trick: S has exactly two -1 entries per reaction, so the mass-action
substrate term exp(log(conc+eps) @ s_neg) is just conc[i1]*conc[i2]. Both
factors are fetched with ONE stacked one-hot gather matmul ([P1|P2].T @ concT
-> [128, nc] PSUM) and multiplied on the vector engine — no Ln/Exp needed, so
the scalar engine only ever runs Sigmoid/Copy from a single activation table
set (avoids ~1.3us table reloads per switch). kcat folds into sigmoid's
multiplier lane... (kcat stays a separate DVE multiply folded into the gather
product via scalar_tensor_tensor), DT folds into the scatter weights (DT*S).

The gene path uses fp16 (G is exactly 0/1; halves its HBM traffic); the
conc path stays fp32 end-to-end.
"""

import threading

import numpy as np

N_METS = 114
N_RXNS = 64
N_GENES = 212
B = 262144
N_CORES = 8
BS = B // N_CORES      # 32768 cells per core
CT = 4096              # cells per SBUF tile (DMA granularity)
NC = 512               # cells per PSUM chunk (one fp32 bank)
G_HI = 128             # gene split: partitions 0..127
G_LO = N_GENES - G_HI  # remaining 84 gene rows
DT = 0.01
EPS = 1e-6

_lock = threading.Lock()
_cached = {}


def _build_program():
    import concourse.mybir as mybir
    import concourse.tile as tile
    from concourse import bacc
    from concourse.alu_op_type import AluOpType

    f32 = mybir.dt.float32
    f16 = mybir.dt.float16
    AF = mybir.ActivationFunctionType

    nc = bacc.Bacc(
        "TRN2", target_bir_lowering=False, debug=False, num_devices=N_CORES
    )
    d_conc = nc.dram_tensor("conc_t", [N_METS, BS], f32, kind="ExternalInput").ap()
    d_ghi = nc.dram_tensor("gene_hi", [G_HI, BS], f16, kind="ExternalInput").ap()
    d_glo = nc.dram_tensor("gene_lo", [G_LO, BS], f16, kind="ExternalInput").ap()
    d_Ghi = nc.dram_tensor("g_map_hi", [G_HI, N_RXNS], f16, kind="ExternalInput").ap()
    d_Glo = nc.dram_tensor("g_map_lo", [G_LO, N_RXNS], f16, kind="ExternalInput").ap()
    # stacked one-hot substrate selectors: col j -> substrate1 of rxn j,
    # col 64+j -> substrate2 of rxn j
    d_psel = nc.dram_tensor("p_sel", [N_METS, 2 * N_RXNS], f32, kind="ExternalInput").ap()
    d_sdt = nc.dram_tensor("s_dt", [N_RXNS, N_METS], f16, kind="ExternalInput").ap()
    d_kc = nc.dram_tensor("kcat_r", [N_RXNS, 1], f32, kind="ExternalInput").ap()
    d_out = nc.dram_tensor("out_t", [N_METS, BS], f32, kind="ExternalOutput").ap()

    with tile.TileContext(nc) as tc:
        with (
            tc.tile_pool(name="consts", bufs=1) as consts,
            tc.tile_pool(name="io", bufs=2) as io,
            tc.tile_pool(name="mid", bufs=3) as mid,
            tc.tile_pool(name="ps_e", bufs=2, space="PSUM") as ps_e,
            tc.tile_pool(name="ps_g", bufs=2, space="PSUM") as ps_g,
            tc.tile_pool(name="ps_d", bufs=2, space="PSUM") as ps_d,
        ):
            c_Ghi = consts.tile([G_HI, N_RXNS], f16)
            nc.sync.dma_start(out=c_Ghi, in_=d_Ghi)
            c_Glo = consts.tile([G_LO, N_RXNS], f16)
            nc.sync.dma_start(out=c_Glo, in_=d_Glo)
            c_psel = consts.tile([N_METS, 2 * N_RXNS], f32)
            nc.sync.dma_start(out=c_psel, in_=d_psel)
            c_sdt = consts.tile([N_RXNS, N_METS], f16)
            nc.sync.dma_start(out=c_sdt, in_=d_sdt)
            c_kc = consts.tile([N_RXNS, 1], f32)
            nc.sync.dma_start(out=c_kc, in_=d_kc)

            for it in range(BS // CT):
                sl = slice(it * CT, (it + 1) * CT)
                t_conc = io.tile([N_METS, CT], f32, tag="conc")
                nc.sync.dma_start(out=t_conc, in_=d_conc[:, sl])
                t_ghi = io.tile([G_HI, CT], f16, tag="ghi")
                nc.sync.dma_start(out=t_ghi, in_=d_ghi[:, sl])
                t_glo = io.tile([G_LO, CT], f16, tag="glo")
                nc.sync.dma_start(out=t_glo, in_=d_glo[:, sl])
                t_out = io.tile([N_METS, CT], f32, tag="out")

                for ic in range(CT // NC):
                    cs = slice(ic * NC, (ic + 1) * NC)
                    # enzyme = gene @ G  (contract genes: 128 + 84)
                    p_e = ps_e.tile([N_RXNS, NC], f32, tag="pe")
                    nc.tensor.matmul(p_e, c_Ghi, t_ghi[:, cs], start=True, stop=False)
                    nc.tensor.matmul(p_e, c_Glo, t_glo[:, cs], start=False, stop=True)
                    # substrate gather: p_g[0:64] = conc[i1], p_g[64:128] = conc[i2]
                    p_g = ps_g.tile([2 * N_RXNS, NC], f32, tag="pg")
                    nc.tensor.matmul(p_g, c_psel, t_conc[:, cs], start=True, stop=True)
                    # sig = kcat-weighted sigmoid? no: plain sigmoid (f16)
                    t_sig = mid.tile([N_RXNS, NC], f16, tag="sig")
                    nc.scalar.activation(t_sig, p_e, AF.Sigmoid)
                    # second substrate factor PSUM -> SBUF (f16)
                    t_g2 = mid.tile([N_RXNS, NC], f16, tag="g2")
                    nc.scalar.activation(t_g2, p_g[N_RXNS:, :], AF.Copy)
                    # prod = (g1 + eps) * g2   (eps matches reference's log(c+eps))
                    t_prod = mid.tile([N_RXNS, NC], f16, tag="prod")
                    nc.vector.scalar_tensor_tensor(
                        t_prod, p_g[:N_RXNS, :], EPS, t_g2,
                        AluOpType.add, AluOpType.mult,
                    )
                    # rates = kcat * sig * prod; kcat folded via per-partition scalar
                    t_rates = mid.tile([N_RXNS, NC], f16, tag="rates")
                    nc.vector.scalar_tensor_tensor(
                        t_rates, t_sig, c_kc, t_prod,
                        AluOpType.mult, AluOpType.mult,
                    )
                    # scatter: DT * (rates @ S.T), DT pre-folded into s_dt
                    p_d = ps_d.tile([N_METS, NC], f32, tag="pd")
                    nc.tensor.matmul(p_d, c_sdt, t_rates, start=True, stop=True)
                    # out = relu(conc + DT*d_conc)
                    nc.vector.tensor_add(t_out[:, cs], t_conc[:, cs], p_d)
                    nc.gpsimd.tensor_scalar_max(t_out[:, cs], t_out[:, cs], 0.0)

                nc.sync.dma_start(out=d_out[:, sl], in_=t_out)

    nc.compile()
    return nc


def _get_program():
    with _lock:
        if "nc" not in _cached:
            _cached["nc"] = _build_program()
        return _cached["nc"]


def _host_consts(S, G, kcat):
    g_f16 = G.astype(np.float16)
    # one-hot substrate selector columns from S (exactly two -1 per reaction)
    p_sel = np.zeros((N_METS, 2 * N_RXNS), dtype=np.float32)
    for j in range(N_RXNS):
        subs = np.where(S[:, j] < 0)[0]
        assert len(subs) == 2, f"reaction {j} has {len(subs)} substrates"
        p_sel[subs[0], j] = 1.0
        p_sel[subs[1], N_RXNS + j] = 1.0
    return {
        "g_map_hi": np.ascontiguousarray(g_f16[:G_HI]),
        "g_map_lo": np.ascontiguousarray(g_f16[G_HI:]),
        "p_sel": p_sel,
        "s_dt": np.ascontiguousarray((DT * S).T.astype(np.float16)),
        "kcat_r": kcat.astype(np.float32).reshape(N_RXNS, 1),
    }


def kernel(conc, gene_expr, S, G, kcat):
    from concourse.bass_utils import run_bass_kernel_spmd

    conc = np.asarray(conc, dtype=np.float32)
    gene_expr = np.asarray(gene_expr, dtype=np.float32)
    S = np.asarray(S, dtype=np.float32)
    G = np.asarray(G, dtype=np.float32)
    kcat = np.asarray(kcat, dtype=np.float32)

    nc = _get_program()
    consts = _host_consts(S, G, kcat)

    in_maps = []
    for c in range(N_CORES):
        rows = slice(c * BS, (c + 1) * BS)
        gene_t = gene_expr[rows].T.astype(np.float16)
        in_maps.append(
            {
                "conc_t": np.ascontiguousarray(conc[rows].T),
                "gene_hi": np.ascontiguousarray(gene_t[:G_HI]),
                "gene_lo": np.ascontiguousarray(gene_t[G_HI:]),
                **consts,
            }
        )

    res = run_bass_kernel_spmd(nc, in_maps, core_ids=list(range(N_CORES)))

    out = np.empty((B, N_METS), dtype=np.float32)
    for c in range(N_CORES):
        out[c * BS : (c + 1) * BS] = res.results[c]["out_t"].T
    return out


# revision 6
# speedup vs baseline: 2.2704x; 1.0140x over previous
"""Trainium2 Bass kernel: gather-rate-scatter metabolite update (one Euler
kinetics step) for B=262144 cells, data-parallel across 8 NeuronCores.

Math (per cell batch):
    enzyme = gene_expr @ G                      [B, 64]
    rates  = kcat * sigmoid(enzyme) * exp(log(conc+eps) @ max(-S,0))
    out    = max(conc + DT * (rates @ S.T), 0)  [B, 114]

Device strategy: shard B across 8 cores (pure data parallelism). On the host
the per-core shards are transposed so features sit on SBUF partitions; the
small contractions then run natively on the tensor engine with zero on-chip
transposes and fully contiguous DMA.

Key tricks:
- S has exactly two -1 entries per reaction, so the mass-action substrate
  term exp(log(conc+eps) @ s_neg) is just conc[i1]*conc[i2]. The factors are
  fetched with two one-hot gather matmuls (float32r: full-rate on the PE at
  N=512, ~19-bit mantissa — exact enough for a gather) and multiplied on the
  vector engine. No Ln/Exp => the scalar engine only runs Sigmoid/Copy from
  one activation-table set (saves a ~1.3us table reload per function switch).
- kcat and DT fold into the scatter weights: d_out = (DT*S*kcat).T @ raw_rates.
- The gene path uses fp16 (G is exactly 0/1; halves its HBM traffic); the
  conc path stays fp32 end-to-end.
"""

import threading

import numpy as np

N_METS = 114
N_RXNS = 64
N_GENES = 212
B = 262144
N_CORES = 8
BS = B // N_CORES      # 32768 cells per core
CT = 4096              # cells per SBUF tile (DMA granularity)
NC = 512               # cells per PSUM chunk (one fp32 bank)
G_HI = 128             # gene split: partitions 0..127
G_LO = N_GENES - G_HI  # remaining 84 gene rows
DT = 0.01

_lock = threading.Lock()
_cached = {}


def _build_program():
    import concourse.mybir as mybir
    import concourse.tile as tile
    from concourse import bacc

    f32 = mybir.dt.float32
    f32r = mybir.dt.float32r
    f16 = mybir.dt.float16
    AF = mybir.ActivationFunctionType

    nc = bacc.Bacc(
        "TRN2", target_bir_lowering=False, debug=False, num_devices=N_CORES
    )
    d_conc = nc.dram_tensor("conc_t", [N_METS, BS], f32r, kind="ExternalInput").ap()
    d_ghi = nc.dram_tensor("gene_hi", [G_HI, BS], f16, kind="ExternalInput").ap()
    d_glo = nc.dram_tensor("gene_lo", [G_LO, BS], f16, kind="ExternalInput").ap()
    d_Ghi = nc.dram_tensor("g_map_hi", [G_HI, N_RXNS], f16, kind="ExternalInput").ap()
    d_Glo = nc.dram_tensor("g_map_lo", [G_LO, N_RXNS], f16, kind="ExternalInput").ap()
    # one-hot substrate selectors: col j of p_sel[:, :64] -> substrate1 of
    # rxn j, col j of p_sel[:, 64:] -> substrate2 of rxn j
    d_psel = nc.dram_tensor("p_sel", [N_METS, 2 * N_RXNS], f32r, kind="ExternalInput").ap()
    # scatter weights with DT and kcat pre-folded: (DT * S * kcat).T
    d_sdt = nc.dram_tensor("s_dtk", [N_RXNS, N_METS], f16, kind="ExternalInput").ap()
    d_out = nc.dram_tensor("out_t", [N_METS, BS], f32, kind="ExternalOutput").ap()

    with tile.TileContext(nc) as tc:
        with (
            tc.tile_pool(name="consts", bufs=1) as consts,
            tc.tile_pool(name="io", bufs=3) as io,
            tc.tile_pool(name="mid", bufs=4) as mid,
            tc.tile_pool(name="ps_e", bufs=2, space="PSUM") as ps_e,
            tc.tile_pool(name="ps_ga", bufs=2, space="PSUM") as ps_ga,
            tc.tile_pool(name="ps_gb", bufs=2, space="PSUM") as ps_gb,
            tc.tile_pool(name="ps_d", bufs=2, space="PSUM") as ps_d,
        ):
            c_Ghi = consts.tile([G_HI, N_RXNS], f16)
            nc.sync.dma_start(out=c_Ghi, in_=d_Ghi)
            c_Glo = consts.tile([G_LO, N_RXNS], f16)
            nc.sync.dma_start(out=c_Glo, in_=d_Glo)
            c_psel = consts.tile([N_METS, 2 * N_RXNS], f32r)
            nc.sync.dma_start(out=c_psel, in_=d_psel)
            c_sdt = consts.tile([N_RXNS, N_METS], f16)
            nc.sync.dma_start(out=c_sdt, in_=d_sdt)

            for it in range(BS // CT):
                sl = slice(it * CT, (it + 1) * CT)
                t_conc = io.tile([N_METS, CT], f32r, tag="conc")
                nc.sync.dma_start(out=t_conc, in_=d_conc[:, sl])
                t_ghi = io.tile([G_HI, CT], f16, tag="ghi")
                nc.sync.dma_start(out=t_ghi, in_=d_ghi[:, sl])
                t_glo = io.tile([G_LO, CT], f16, tag="glo")
                nc.sync.dma_start(out=t_glo, in_=d_glo[:, sl])
                t_out = io.tile([N_METS, CT], f32, tag="out")

                for ic in range(CT // NC):
                    cs = slice(ic * NC, (ic + 1) * NC)
                    # enzyme = gene @ G  (contract genes: 128 + 84)
                    p_e = ps_e.tile([N_RXNS, NC], f32, tag="pe")
                    nc.tensor.matmul(p_e, c_Ghi, t_ghi[:, cs], start=True, stop=False)
                    nc.tensor.matmul(p_e, c_Glo, t_glo[:, cs], start=False, stop=True)
                    # substrate factor gathers (f32r = full-rate fp32 matmul)
                    rhs = t_conc[:, cs]
                    p_ga = ps_ga.tile([N_RXNS, NC], f32, tag="pga")
                    nc.tensor.matmul(
                        p_ga, c_psel[:, :N_RXNS], rhs, start=True, stop=True
                    )
                    p_gb = ps_gb.tile([N_RXNS, NC], f32, tag="pgb")
                    nc.tensor.matmul(
                        p_gb, c_psel[:, N_RXNS:], rhs, start=True, stop=True
                    )
                    t_sig = mid.tile([N_RXNS, NC], f16, tag="sig")
                    nc.scalar.activation(t_sig, p_e, AF.Sigmoid)
                    t_g2 = mid.tile([N_RXNS, NC], f16, tag="g2")
                    nc.scalar.activation(t_g2, p_gb, AF.Copy)
                    # prod = conc[i1] * conc[i2]
                    t_prod = mid.tile([N_RXNS, NC], f16, tag="prod")
                    nc.vector.tensor_mul(t_prod, p_ga, t_g2)
                    # raw rates (kcat folded into scatter weights)
                    t_rates = mid.tile([N_RXNS, NC], f16, tag="rates")
                    nc.vector.tensor_mul(t_rates, t_sig, t_prod)
                    # scatter: conc + (DT*kcat*S) @ rates, then relu
                    p_d = ps_d.tile([N_METS, NC], f32, tag="pd")
                    nc.tensor.matmul(p_d, c_sdt, t_rates, start=True, stop=True)
                    nc.vector.tensor_add(t_out[:, cs], t_conc[:, cs].bitcast(f32), p_d)
                    nc.gpsimd.tensor_scalar_max(t_out[:, cs], t_out[:, cs], 0.0)

                nc.sync.dma_start(out=d_out[:, sl], in_=t_out)

    nc.compile()
    return nc


def _get_program():
    with _lock:
        if "nc" not in _cached:
            _cached["nc"] = _build_program()
        return _cached["nc"]


def _host_consts(S, G, kcat):
    g_f16 = G.astype(np.float16)
    # one-hot substrate selector columns from S (exactly two -1 per reaction)
    p_sel = np.zeros((N_METS, 2 * N_RXNS), dtype=np.float32)
    for j in range(N_RXNS):
        subs = np.where(S[:, j] < 0)[0]
        assert len(subs) == 2, f"reaction {j} has {len(subs)} substrates"
        p_sel[subs[0], j] = 1.0
        p_sel[subs[1], N_RXNS + j] = 1.0
    return {
        "g_map_hi": np.ascontiguousarray(g_f16[:G_HI]),
        "g_map_lo": np.ascontiguousarray(g_f16[G_HI:]),
        "p_sel": p_sel,
        "s_dtk": np.ascontiguousarray(
            (DT * S * kcat[None, :]).T.astype(np.float16)
        ),
    }


def kernel(conc, gene_expr, S, G, kcat):
    from concourse.bass_utils import run_bass_kernel_spmd

    conc = np.asarray(conc, dtype=np.float32)
    gene_expr = np.asarray(gene_expr, dtype=np.float32)
    S = np.asarray(S, dtype=np.float32)
    G = np.asarray(G, dtype=np.float32)
    kcat = np.asarray(kcat, dtype=np.float32)

    nc = _get_program()
    consts = _host_consts(S, G, kcat)

    in_maps = []
    for c in range(N_CORES):
        rows = slice(c * BS, (c + 1) * BS)
        gene_t = gene_expr[rows].T.astype(np.float16)
        in_maps.append(
            {
                "conc_t": np.ascontiguousarray(conc[rows].T),
                "gene_hi": np.ascontiguousarray(gene_t[:G_HI]),
                "gene_lo": np.ascontiguousarray(gene_t[G_HI:]),
                **consts,
            }
        )

    res = run_bass_kernel_spmd(nc, in_maps, core_ids=list(range(N_CORES)))

    out = np.empty((B, N_METS), dtype=np.float32)
    for c in range(N_CORES):
        out[c * BS : (c + 1) * BS] = res.results[c]["out_t"].T
    return out


# revision 7
# speedup vs baseline: 2.7710x; 1.2205x over previous
"""Trainium2 Bass kernel: gather-rate-scatter metabolite update (one Euler
kinetics step) for B=262144 cells, data-parallel across 8 NeuronCores.

Math (per cell batch):
    enzyme = gene_expr @ G                      [B, 64]
    rates  = kcat * sigmoid(enzyme) * exp(log(conc+eps) @ max(-S,0))
    out    = max(conc + DT * (rates @ S.T), 0)  [B, 114]

Device strategy: shard B across 8 cores (pure data parallelism). On the host
the per-core shards are transposed so features sit on SBUF partitions; the
small contractions then run natively on the tensor engine with zero on-chip
transposes and fully contiguous DMA.

Key tricks:
- S has exactly two -1 entries per reaction, so the mass-action substrate
  term exp(log(conc+eps) @ s_neg) is just conc[i1]*conc[i2]. The factors are
  fetched with two one-hot gather matmuls (float32r: full-rate on the PE at
  N=512, ~19-bit mantissa — exact enough for a gather) and multiplied on the
  vector engine. No Ln/Exp => the scalar engine only runs Sigmoid/Copy from
  one activation-table set (saves a ~1.3us table reload per function switch).
- kcat and DT fold into the scatter weights: d_out = (DT*S*kcat).T @ raw_rates.
- The gene path uses fp16 (G is exactly 0/1; halves its HBM traffic); the
  conc path stays fp32 end-to-end.
"""

import threading

import numpy as np

N_METS = 114
N_RXNS = 64
N_GENES = 212
B = 262144
N_CORES = 8
BS = B // N_CORES      # 32768 cells per core
CT = 4096              # cells per SBUF tile (DMA granularity)
NC = 512               # cells per PSUM chunk (one fp32 bank)
G_HI = 128             # gene split: partitions 0..127
G_LO = N_GENES - G_HI  # remaining 84 gene rows
DT = 0.01

_lock = threading.Lock()
_cached = {}


def _build_program():
    import concourse.mybir as mybir
    import concourse.tile as tile
    from concourse import bacc

    f32 = mybir.dt.float32
    f32r = mybir.dt.float32r
    f16 = mybir.dt.float16
    AF = mybir.ActivationFunctionType

    nc = bacc.Bacc(
        "TRN2", target_bir_lowering=False, debug=False, num_devices=N_CORES
    )
    d_conc = nc.dram_tensor("conc_t", [N_METS, BS], f32r, kind="ExternalInput").ap()
    d_ghi = nc.dram_tensor("gene_hi", [G_HI, BS], f16, kind="ExternalInput").ap()
    d_glo = nc.dram_tensor("gene_lo", [G_LO, BS], f16, kind="ExternalInput").ap()
    d_Ghi = nc.dram_tensor("g_map_hi", [G_HI, N_RXNS], f16, kind="ExternalInput").ap()
    d_Glo = nc.dram_tensor("g_map_lo", [G_LO, N_RXNS], f16, kind="ExternalInput").ap()
    # one-hot substrate selectors: col j of p_sel[:, :64] -> substrate1 of
    # rxn j, col j of p_sel[:, 64:] -> substrate2 of rxn j
    d_psel = nc.dram_tensor("p_sel", [N_METS, 2 * N_RXNS], f32r, kind="ExternalInput").ap()
    # scatter weights with DT and kcat pre-folded: (DT * S * kcat).T
    d_sdt = nc.dram_tensor("s_dtk", [N_RXNS, N_METS], f16, kind="ExternalInput").ap()
    d_out = nc.dram_tensor("out_t", [N_METS, BS], f32, kind="ExternalOutput").ap()

    with tile.TileContext(nc) as tc:
        with (
            tc.tile_pool(name="consts", bufs=1) as consts,
            tc.tile_pool(name="io", bufs=3) as io,
            tc.tile_pool(name="mid", bufs=4) as mid,
            tc.tile_pool(name="ps_e", bufs=2, space="PSUM") as ps_e,
            tc.tile_pool(name="ps_ga", bufs=2, space="PSUM") as ps_ga,
            tc.tile_pool(name="ps_gb", bufs=2, space="PSUM") as ps_gb,
            tc.tile_pool(name="ps_d", bufs=2, space="PSUM") as ps_d,
        ):
            c_Ghi = consts.tile([G_HI, N_RXNS], f16)
            nc.sync.dma_start(out=c_Ghi, in_=d_Ghi)
            c_Glo = consts.tile([G_LO, N_RXNS], f16)
            nc.sync.dma_start(out=c_Glo, in_=d_Glo)
            c_psel = consts.tile([N_METS, 2 * N_RXNS], f32r)
            nc.sync.dma_start(out=c_psel, in_=d_psel)
            c_sdt = consts.tile([N_RXNS, N_METS], f16)
            nc.sync.dma_start(out=c_sdt, in_=d_sdt)

            for it in range(BS // CT):
                sl = slice(it * CT, (it + 1) * CT)
                t_conc = io.tile([N_METS, CT], f32r, tag="conc")
                nc.sync.dma_start(out=t_conc, in_=d_conc[:, sl])
                t_ghi = io.tile([G_HI, CT], f16, tag="ghi")
                nc.sync.dma_start(out=t_ghi, in_=d_ghi[:, sl])
                t_glo = io.tile([G_LO, CT], f16, tag="glo")
                nc.sync.dma_start(out=t_glo, in_=d_glo[:, sl])
                t_out = io.tile([N_METS, CT], f32, tag="out")

                for ic in range(CT // NC):
                    cs = slice(ic * NC, (ic + 1) * NC)
                    # enzyme = gene @ G  (contract genes: 128 + 84)
                    p_e = ps_e.tile([N_RXNS, NC], f32, tag="pe")
                    nc.tensor.matmul(p_e, c_Ghi, t_ghi[:, cs], start=True, stop=False)
                    nc.tensor.matmul(p_e, c_Glo, t_glo[:, cs], start=False, stop=True)
                    # substrate factor gathers (f32r = full-rate fp32 matmul)
                    rhs = t_conc[:, cs]
                    p_ga = ps_ga.tile([N_RXNS, NC], f32, tag="pga")
                    nc.tensor.matmul(
                        p_ga, c_psel[:, :N_RXNS], rhs, start=True, stop=True
                    )
                    p_gb = ps_gb.tile([N_RXNS, NC], f32, tag="pgb")
                    nc.tensor.matmul(
                        p_gb, c_psel[:, N_RXNS:], rhs, start=True, stop=True
                    )
                    t_sig = mid.tile([N_RXNS, NC], f16, tag="sig")
                    nc.scalar.activation(t_sig, p_e, AF.Sigmoid)
                    t_g2 = mid.tile([N_RXNS, NC], f16, tag="g2")
                    nc.scalar.activation(t_g2, p_gb, AF.Copy)
                    # prod = conc[i1] * conc[i2]
                    t_prod = mid.tile([N_RXNS, NC], f16, tag="prod")
                    nc.vector.tensor_mul(t_prod, p_ga, t_g2)
                    # raw rates (kcat folded into scatter weights)
                    t_rates = mid.tile([N_RXNS, NC], f16, tag="rates")
                    nc.vector.tensor_mul(t_rates, t_sig, t_prod)
                    # scatter: conc + (DT*kcat*S) @ rates, then relu
                    p_d = ps_d.tile([N_METS, NC], f32, tag="pd")
                    nc.tensor.matmul(p_d, c_sdt, t_rates, start=True, stop=True)
                    nc.vector.tensor_add(t_out[:, cs], t_conc[:, cs].bitcast(f32), p_d)
                    nc.gpsimd.tensor_scalar_max(t_out[:, cs], t_out[:, cs], 0.0)

                # store from the Pool engine (SWDGE) so a store waiting on the
                # compute tail never head-of-line-blocks the SP sequencer's
                # next-tile loads
                nc.gpsimd.dma_start(out=d_out[:, sl], in_=t_out)

    nc.compile()
    return nc


def _get_program():
    with _lock:
        if "nc" not in _cached:
            _cached["nc"] = _build_program()
        return _cached["nc"]


def _host_consts(S, G, kcat):
    g_f16 = G.astype(np.float16)
    # one-hot substrate selector columns from S (exactly two -1 per reaction)
    p_sel = np.zeros((N_METS, 2 * N_RXNS), dtype=np.float32)
    for j in range(N_RXNS):
        subs = np.where(S[:, j] < 0)[0]
        assert len(subs) == 2, f"reaction {j} has {len(subs)} substrates"
        p_sel[subs[0], j] = 1.0
        p_sel[subs[1], N_RXNS + j] = 1.0
    return {
        "g_map_hi": np.ascontiguousarray(g_f16[:G_HI]),
        "g_map_lo": np.ascontiguousarray(g_f16[G_HI:]),
        "p_sel": p_sel,
        "s_dtk": np.ascontiguousarray(
            (DT * S * kcat[None, :]).T.astype(np.float16)
        ),
    }


def kernel(conc, gene_expr, S, G, kcat):
    from concourse.bass_utils import run_bass_kernel_spmd

    conc = np.asarray(conc, dtype=np.float32)
    gene_expr = np.asarray(gene_expr, dtype=np.float32)
    S = np.asarray(S, dtype=np.float32)
    G = np.asarray(G, dtype=np.float32)
    kcat = np.asarray(kcat, dtype=np.float32)

    nc = _get_program()
    consts = _host_consts(S, G, kcat)

    in_maps = []
    for c in range(N_CORES):
        rows = slice(c * BS, (c + 1) * BS)
        gene_t = gene_expr[rows].T.astype(np.float16)
        in_maps.append(
            {
                "conc_t": np.ascontiguousarray(conc[rows].T),
                "gene_hi": np.ascontiguousarray(gene_t[:G_HI]),
                "gene_lo": np.ascontiguousarray(gene_t[G_HI:]),
                **consts,
            }
        )

    res = run_bass_kernel_spmd(nc, in_maps, core_ids=list(range(N_CORES)))

    out = np.empty((B, N_METS), dtype=np.float32)
    for c in range(N_CORES):
        out[c * BS : (c + 1) * BS] = res.results[c]["out_t"].T
    return out


# revision 15
# speedup vs baseline: 4.2351x; 1.5283x over previous
"""Trainium2 Bass kernel: gather-rate-scatter metabolite update (one Euler
kinetics step) for B=262144 cells, data-parallel across 8 NeuronCores.

Math (per cell batch):
    enzyme = gene_expr @ G                      [B, 64]
    rates  = kcat * sigmoid(enzyme) * exp(log(conc+eps) @ max(-S,0))
    out    = max(conc + DT * (rates @ S.T), 0)  [B, 114]

Device strategy: shard B across 8 cores (pure data parallelism). On the host
the per-core shards are transposed so features sit on SBUF partitions; the
small contractions then run natively on the tensor engine with zero on-chip
transposes and fully contiguous DMA.

Key tricks:
- S has exactly two -1 entries per reaction, so the mass-action substrate
  term exp(log(conc+eps) @ s_neg) is just conc[i1]*conc[i2]. The factors are
  fetched with two one-hot gather matmuls (float32r: full-rate on the PE at
  N=512, ~19-bit mantissa — exact enough for a gather) and multiplied on the
  vector engine. No Ln/Exp => the scalar engine only runs Sigmoid/Copy from
  one activation-table set (saves a ~1.3us table reload per function switch).
- kcat and DT fold into the scatter weights: d_out = (DT*S*kcat).T @ raw_rates.
- The gene path uses fp16 (G is exactly 0/1; halves its HBM traffic); the
  conc path stays fp32 end-to-end.
"""

import threading

import numpy as np

N_METS = 114
N_RXNS = 64
N_GENES = 212
B = 262144
N_CORES = 8
BS = B // N_CORES      # 32768 cells per core
CT = 4096              # cells per SBUF tile (DMA granularity)
NC = 512               # cells per PSUM chunk (one fp32 bank)
G_HI = 128             # gene split: partitions 0..127
G_LO = N_GENES - G_HI  # remaining 84 gene rows
DT = 0.01

_lock = threading.Lock()
_cached = {}


def _build_program(ct=CT, io_bufs=4, mid_bufs=4, taper=(), gene_engine=None, pe_bufs=2, pd_bufs=2, conc_f16=True):
    import concourse.mybir as mybir
    import concourse.tile as tile
    from concourse import bacc

    f32 = mybir.dt.float32
    f32r = mybir.dt.float32r
    f16 = mybir.dt.float16
    AF = mybir.ActivationFunctionType

    nc = bacc.Bacc(
        "TRN2", target_bir_lowering=False, debug=False, num_devices=N_CORES
    )
    conc_dt = f16 if conc_f16 else f32r
    d_conc = nc.dram_tensor("conc_t", [N_METS, BS], conc_dt, kind="ExternalInput").ap()
    d_ghi = nc.dram_tensor("gene_hi", [G_HI, BS], f16, kind="ExternalInput").ap()
    d_glo = nc.dram_tensor("gene_lo", [G_LO, BS], f16, kind="ExternalInput").ap()
    d_Ghi = nc.dram_tensor("g_map_hi", [G_HI, N_RXNS], f16, kind="ExternalInput").ap()
    d_Glo = nc.dram_tensor("g_map_lo", [G_LO, N_RXNS], f16, kind="ExternalInput").ap()
    # one-hot substrate selectors: col j of p_sel[:, :64] -> substrate1 of
    # rxn j, col j of p_sel[:, 64:] -> substrate2 of rxn j
    d_psel = nc.dram_tensor("p_sel", [N_METS, 2 * N_RXNS], conc_dt, kind="ExternalInput").ap()
    # scatter weights with DT and kcat pre-folded: (DT * S * kcat).T
    # duplicated on both partition halves so mm_d can consume rates that
    # live at base partition 0 or 64 (lhsT/rhs must share base_partition)
    d_sdt = nc.dram_tensor("s_dtk", [2 * N_RXNS, N_METS], f16, kind="ExternalInput").ap()
    out_dt = f16
    d_out = nc.dram_tensor("out_t", [N_METS, BS], out_dt, kind="ExternalOutput").ap()

    with tile.TileContext(nc) as tc:
        with (
            tc.tile_pool(name="consts", bufs=1) as consts,
            tc.tile_pool(name="io", bufs=io_bufs) as io,
            tc.tile_pool(name="mid", bufs=mid_bufs) as mid,
            tc.tile_pool(name="ps_e", bufs=pe_bufs, space="PSUM") as ps_e,
            tc.tile_pool(name="ps_ga", bufs=2, space="PSUM") as ps_ga,
            tc.tile_pool(name="ps_gb", bufs=2, space="PSUM") as ps_gb,
            tc.tile_pool(name="ps_d", bufs=pd_bufs, space="PSUM") as ps_d,
        ):
            c_Ghi = consts.tile([G_HI, N_RXNS], f16)
            nc.sync.dma_start(out=c_Ghi, in_=d_Ghi)
            c_Glo = consts.tile([G_LO, N_RXNS], f16)
            nc.sync.dma_start(out=c_Glo, in_=d_Glo)
            c_psel = consts.tile([N_METS, 2 * N_RXNS], conc_dt)
            nc.sync.dma_start(out=c_psel, in_=d_psel)
            c_sdt = consts.tile([2 * N_RXNS, N_METS], f16)
            nc.sync.dma_start(out=c_sdt, in_=d_sdt)

            taper_cells = sum(taper)
            assert (BS - taper_cells) % ct == 0
            tile_sizes = [ct] * ((BS - taper_cells) // ct) + list(taper)
            tile_starts = [sum(tile_sizes[:i]) for i in range(len(tile_sizes))]
            for it, (t0, tsz) in enumerate(zip(tile_starts, tile_sizes)):
                sl = slice(t0, t0 + tsz)
                gene_eng = nc.sync if gene_engine is None else getattr(nc, gene_engine)
                t_conc = io.tile([N_METS, ct], conc_dt, tag="conc")
                nc.sync.dma_start(out=t_conc[:, :tsz], in_=d_conc[:, sl])
                t_ghi = io.tile([G_HI, ct], f16, tag="ghi")
                gene_eng.dma_start(out=t_ghi[:, :tsz], in_=d_ghi[:, sl])
                t_glo = io.tile([G_LO, ct], f16, tag="glo")
                gene_eng.dma_start(out=t_glo[:, :tsz], in_=d_glo[:, sl])
                t_out = io.tile([N_METS, ct], out_dt, tag="out")

                for ip in range(tsz // (2 * NC)):
                    cs0 = slice((2 * ip) * NC, (2 * ip + 1) * NC)
                    cs1 = slice((2 * ip + 1) * NC, (2 * ip + 2) * NC)
                    # two 512-cell chunks share each PSUM bank: chunk0 on
                    # partitions 0:64, chunk1 on 64:128 — post-matmul ops then
                    # process both chunks in one instruction
                    p_e = ps_e.tile([2 * N_RXNS, NC], f32, tag="pe")
                    p_ga = ps_ga.tile([2 * N_RXNS, NC], f32, tag="pga")
                    p_gb = ps_gb.tile([2 * N_RXNS, NC], f32, tag="pgb")
                    for s, cs in enumerate((cs0, cs1)):
                        half = slice(s * N_RXNS, (s + 1) * N_RXNS)
                        nc.tensor.matmul(p_e[half], c_Ghi, t_ghi[:, cs], start=True, stop=False)
                        nc.tensor.matmul(p_e[half], c_Glo, t_glo[:, cs], start=False, stop=True)
                        nc.tensor.matmul(p_ga[half], c_psel[:, :N_RXNS], t_conc[:, cs], start=True, stop=True)
                        nc.tensor.matmul(p_gb[half], c_psel[:, N_RXNS:], t_conc[:, cs], start=True, stop=True)
                    t_sig = mid.tile([2 * N_RXNS, NC], f16, tag="sig")
                    nc.scalar.activation(t_sig, p_e, AF.Sigmoid)
                    t_g2 = mid.tile([2 * N_RXNS, NC], f16, tag="g2")
                    nc.scalar.activation(t_g2, p_gb, AF.Copy)
                    t_prod = mid.tile([2 * N_RXNS, NC], f16, tag="prod")
                    nc.vector.tensor_mul(t_prod, p_ga, t_g2)
                    t_rates = mid.tile([2 * N_RXNS, NC], f16, tag="rates")
                    nc.vector.tensor_mul(t_rates, t_sig, t_prod)
                    for s, cs in enumerate((cs0, cs1)):
                        half = slice(s * N_RXNS, (s + 1) * N_RXNS)
                        p_d = ps_d.tile([N_METS, NC], f32, tag="pd")
                        nc.tensor.matmul(p_d, c_sdt[half], t_rates[half], start=True, stop=True)
                        # fp16 delta out; host adds conc and clamps
                        if s == 0:
                            nc.vector.tensor_copy(t_out[:, cs], p_d)
                        else:
                            nc.scalar.activation(t_out[:, cs], p_d, AF.Copy)

                # store from the Pool engine (SWDGE) so a store waiting on the
                # compute tail never head-of-line-blocks the SP sequencer's
                # next-tile loads
                nc.gpsimd.dma_start(out=d_out[:, sl], in_=t_out[:, :tsz])

    nc.compile()
    return nc


def _get_program():
    with _lock:
        if "nc" not in _cached:
            _cached["nc"] = _build_program(ct=2048, io_bufs=4)
        return _cached["nc"]


def _host_consts(S, G, kcat):
    g_f16 = G.astype(np.float16)
    # one-hot substrate selector columns from S (exactly two -1 per reaction)
    p_sel = np.zeros((N_METS, 2 * N_RXNS), dtype=np.float32)
    for j in range(N_RXNS):
        subs = np.where(S[:, j] < 0)[0]
        assert len(subs) == 2, f"reaction {j} has {len(subs)} substrates"
        p_sel[subs[0], j] = 1.0
        p_sel[subs[1], N_RXNS + j] = 1.0
    return {
        "g_map_hi": np.ascontiguousarray(g_f16[:G_HI]),
        "g_map_lo": np.ascontiguousarray(g_f16[G_HI:]),
        "p_sel": p_sel.astype(np.float16),
        "s_dtk": np.ascontiguousarray(
            np.vstack([(DT * S * kcat[None, :]).T.astype(np.float16)] * 2)
        ),
    }


def kernel(conc, gene_expr, S, G, kcat):
    from concourse.bass_utils import run_bass_kernel_spmd

    conc = np.asarray(conc, dtype=np.float32)
    gene_expr = np.asarray(gene_expr, dtype=np.float32)
    S = np.asarray(S, dtype=np.float32)
    G = np.asarray(G, dtype=np.float32)
    kcat = np.asarray(kcat, dtype=np.float32)

    nc = _get_program()
    consts = _host_consts(S, G, kcat)

    in_maps = []
    for c in range(N_CORES):
        rows = slice(c * BS, (c + 1) * BS)
        gene_t = gene_expr[rows].T.astype(np.float16)
        in_maps.append(
            {
                "conc_t": np.ascontiguousarray(conc[rows].T.astype(np.float16)),
                "gene_hi": np.ascontiguousarray(gene_t[:G_HI]),
                "gene_lo": np.ascontiguousarray(gene_t[G_HI:]),
                **consts,
            }
        )

    res = run_bass_kernel_spmd(nc, in_maps, core_ids=list(range(N_CORES)))

    # device ships the fp16 delta DT*d_conc; finish out = relu(conc + delta)
    # in fp32 on the host (keeps conc's full precision in the dominant term)
    out = np.empty((B, N_METS), dtype=np.float32)
    for c in range(N_CORES):
        rows = slice(c * BS, (c + 1) * BS)
        delta = res.results[c]["out_t"].T.astype(np.float32)
        np.maximum(conc[rows] + delta, 0.0, out=out[rows])
    return out


# revision 18
# speedup vs baseline: 5.1296x; 1.2112x over previous
"""Trainium2 Bass kernel: gather-rate-scatter metabolite update (one Euler
kinetics step) for B=262144 cells, data-parallel across 8 NeuronCores.

Math (per cell batch):
    enzyme = gene_expr @ G                      [B, 64]
    rates  = kcat * sigmoid(enzyme) * exp(log(conc+eps) @ max(-S,0))
    out    = max(conc + DT * (rates @ S.T), 0)  [B, 114]

Strategy
--------
- Pure data parallelism: B sharded across 8 cores; S/G/kcat replicated.
- Host-side layout prep: per-core shards are transposed so features sit on
  SBUF partitions. The small contractions then run natively on the tensor
  engine with zero on-chip transposes and fully contiguous DMA.
- Graph sparsity pruning (computed from S/G at run time): only genes that
  catalyze some reaction (~98/212), metabolites that are substrates (~74/114)
  and metabolites touched by the scatter (~100/114) are shipped to the
  device. Unused rows contribute exactly nothing.
- S has exactly two -1 entries per reaction, so the mass-action substrate
  term exp(log(conc+eps) @ s_neg) is conc[i1]*conc[i2]: both factors come
  from one-hot gather matmuls and one vector multiply — no Ln/Exp, so the
  scalar engine only runs Sigmoid/Copy out of a single activation-table set
  (avoids ~1.3us table reloads per function switch).
- kcat and DT fold into the scatter weights: delta = (DT*S*kcat).T @ rates.
- PSUM bank pairing: two 512-cell chunks share each 128-partition PSUM bank
  (chunk A on partitions 0:64, chunk B on 64:128), so every post-matmul
  vector/scalar op processes two chunks per instruction.
- The device ships the fp16 delta (DT * d_conc, magnitude ~0.1); the final
  out = relu(conc + delta) runs on the host in fp32 during unsharding. This
  keeps conc's full fp32 precision in the dominant term (strictly more
  accurate than an fp16 output) and halves the store traffic.
- Engine balance under the HBM roofline: loads issue from SP HWDGE, stores
  from Pool SWDGE (no head-of-line blocking of loads), the delta copy
  alternates DVE/ACT.
"""

import threading

import numpy as np

N_METS = 114
N_RXNS = 64
N_GENES = 212
B = 262144
N_CORES = 8
BS = B // N_CORES      # 32768 cells per core
CT = 2048              # cells per SBUF tile (DMA granularity)
NC = 512               # cells per PSUM chunk (half a paired fp32 bank)
DT = 0.01

_lock = threading.Lock()
_cached = {}


def _build_program(ng, ns, nt, ct=CT, io_bufs=4, mid_bufs=4, pe_bufs=2, pd_bufs=2):
    """ng = #used genes, ns = #substrate mets, nt = #scatter-touched mets."""
    import concourse.mybir as mybir
    import concourse.tile as tile
    from concourse import bacc

    f32 = mybir.dt.float32
    f16 = mybir.dt.float16
    AF = mybir.ActivationFunctionType

    # gene rows are loaded in <=128-partition groups (ng=98 here -> one)
    g_groups = [(g0, min(g0 + 128, ng)) for g0 in range(0, ng, 128)]

    nc = bacc.Bacc(
        "TRN2", target_bir_lowering=False, debug=False, num_devices=N_CORES
    )
    d_conc = nc.dram_tensor("conc_t", [ns, BS], f16, kind="ExternalInput").ap()
    d_gene = [
        nc.dram_tensor(f"gene_{i}", [g1 - g0, BS], f16, kind="ExternalInput").ap()
        for i, (g0, g1) in enumerate(g_groups)
    ]
    d_G = [
        nc.dram_tensor(f"g_map_{i}", [g1 - g0, N_RXNS], f16, kind="ExternalInput").ap()
        for i, (g0, g1) in enumerate(g_groups)
    ]
    # one-hot substrate selectors over the pruned met axis: col j ->
    # substrate1 of rxn j, col 64+j -> substrate2 of rxn j
    d_psel = nc.dram_tensor("p_sel", [ns, 2 * N_RXNS], f16, kind="ExternalInput").ap()
    # scatter weights with DT and kcat folded, over touched mets; duplicated
    # on both partition halves so mm_d can consume rates at base 0 or 64
    d_sdt = nc.dram_tensor("s_dtk", [2 * N_RXNS, nt], f16, kind="ExternalInput").ap()
    d_out = nc.dram_tensor("out_t", [nt, BS], f16, kind="ExternalOutput").ap()

    with tile.TileContext(nc) as tc:
        with (
            tc.tile_pool(name="consts", bufs=1) as consts,
            tc.tile_pool(name="io", bufs=io_bufs) as io,
            tc.tile_pool(name="mid", bufs=mid_bufs) as mid,
            tc.tile_pool(name="ps_e", bufs=pe_bufs, space="PSUM") as ps_e,
            tc.tile_pool(name="ps_ga", bufs=2, space="PSUM") as ps_ga,
            tc.tile_pool(name="ps_gb", bufs=2, space="PSUM") as ps_gb,
            tc.tile_pool(name="ps_d", bufs=pd_bufs, space="PSUM") as ps_d,
        ):
            c_G = []
            for i, (g0, g1) in enumerate(g_groups):
                t = consts.tile([g1 - g0, N_RXNS], f16, name=f"c_G{i}")
                nc.sync.dma_start(out=t, in_=d_G[i])
                c_G.append(t)
            c_psel = consts.tile([ns, 2 * N_RXNS], f16)
            nc.sync.dma_start(out=c_psel, in_=d_psel)
            c_sdt = consts.tile([2 * N_RXNS, nt], f16)
            nc.sync.dma_start(out=c_sdt, in_=d_sdt)

            for it in range(BS // ct):
                sl = slice(it * ct, (it + 1) * ct)
                t_conc = io.tile([ns, ct], f16, tag="conc")
                nc.sync.dma_start(out=t_conc, in_=d_conc[:, sl])
                t_gene = []
                for i, (g0, g1) in enumerate(g_groups):
                    t = io.tile([g1 - g0, ct], f16, tag=f"gene{i}", name=f"t_gene{i}")
                    nc.sync.dma_start(out=t, in_=d_gene[i][:, sl])
                    t_gene.append(t)
                t_out = io.tile([nt, ct], f16, tag="out")

                for ip in range(ct // (2 * NC)):
                    cs0 = slice((2 * ip) * NC, (2 * ip + 1) * NC)
                    cs1 = slice((2 * ip + 1) * NC, (2 * ip + 2) * NC)
                    # two 512-cell chunks share each PSUM bank (partitions
                    # 0:64 and 64:128) so post-matmul ops cover both at once
                    p_e = ps_e.tile([2 * N_RXNS, NC], f32, tag="pe")
                    p_ga = ps_ga.tile([2 * N_RXNS, NC], f32, tag="pga")
                    p_gb = ps_gb.tile([2 * N_RXNS, NC], f32, tag="pgb")
                    for s, cs in enumerate((cs0, cs1)):
                        half = slice(s * N_RXNS, (s + 1) * N_RXNS)
                        for i, t in enumerate(t_gene):
                            nc.tensor.matmul(
                                p_e[half], c_G[i], t[:, cs],
                                start=(i == 0), stop=(i == len(t_gene) - 1),
                            )
                        nc.tensor.matmul(
                            p_ga[half], c_psel[:, :N_RXNS], t_conc[:, cs],
                            start=True, stop=True,
                        )
                        nc.tensor.matmul(
                            p_gb[half], c_psel[:, N_RXNS:], t_conc[:, cs],
                            start=True, stop=True,
                        )
                    t_sig = mid.tile([2 * N_RXNS, NC], f16, tag="sig")
                    nc.scalar.activation(t_sig, p_e, AF.Sigmoid)
                    t_g2 = mid.tile([2 * N_RXNS, NC], f16, tag="g2")
                    nc.scalar.activation(t_g2, p_gb, AF.Copy)
                    t_prod = mid.tile([2 * N_RXNS, NC], f16, tag="prod")
                    nc.vector.tensor_mul(t_prod, p_ga, t_g2)
                    t_rates = mid.tile([2 * N_RXNS, NC], f16, tag="rates")
                    nc.vector.tensor_mul(t_rates, t_sig, t_prod)
                    for s, cs in enumerate((cs0, cs1)):
                        half = slice(s * N_RXNS, (s + 1) * N_RXNS)
                        p_d = ps_d.tile([nt, NC], f32, tag="pd")
                        nc.tensor.matmul(
                            p_d, c_sdt[half], t_rates[half], start=True, stop=True
                        )
                        # fp16 delta out; host adds conc and clamps. The copy
                        # alternates DVE/ACT to balance both engines.
                        if s == 0:
                            nc.vector.tensor_copy(t_out[:, cs], p_d)
                        else:
                            nc.scalar.activation(t_out[:, cs], p_d, AF.Copy)

                # store from the Pool engine (SWDGE) so a store waiting on the
                # compute tail never head-of-line-blocks the SP loads
                nc.gpsimd.dma_start(out=d_out[:, sl], in_=t_out)

    nc.compile()
    return nc


def _get_program(ng, ns, nt):
    key = (ng, ns, nt)
    with _lock:
        if key not in _cached:
            _cached[key] = _build_program(ng, ns, nt)
        return _cached[key]


def _graph_consts(S, G, kcat):
    """Analyze the (replicated, tiny) graph tensors and build device consts."""
    used_genes = np.where(G.any(axis=1))[0]
    used_subs = np.where((S < 0).any(axis=1))[0]
    touched = np.where((S != 0).any(axis=1))[0]
    ng, ns = len(used_genes), len(used_subs)

    sub_pos = {m: i for i, m in enumerate(used_subs)}
    p_sel = np.zeros((ns, 2 * N_RXNS), dtype=np.float16)
    for j in range(N_RXNS):
        subs = np.where(S[:, j] < 0)[0]
        assert len(subs) == 2, f"reaction {j} has {len(subs)} substrates"
        p_sel[sub_pos[subs[0]], j] = 1.0
        p_sel[sub_pos[subs[1]], N_RXNS + j] = 1.0

    sdtk = (DT * S[touched] * kcat[None, :]).T.astype(np.float16)  # [64, nt]
    g_used = G[used_genes].astype(np.float16)                      # [ng, 64]

    consts = {"p_sel": p_sel, "s_dtk": np.ascontiguousarray(np.vstack([sdtk, sdtk]))}
    for i, g0 in enumerate(range(0, ng, 128)):
        consts[f"g_map_{i}"] = np.ascontiguousarray(g_used[g0 : g0 + 128])
    return consts, used_genes, used_subs, touched


def kernel(conc, gene_expr, S, G, kcat):
    from concourse.bass_utils import run_bass_kernel_spmd

    conc = np.asarray(conc, dtype=np.float32)
    gene_expr = np.asarray(gene_expr, dtype=np.float32)
    S = np.asarray(S, dtype=np.float32)
    G = np.asarray(G, dtype=np.float32)
    kcat = np.asarray(kcat, dtype=np.float32)

    consts, used_genes, used_subs, touched = _graph_consts(S, G, kcat)
    ng, ns, nt = len(used_genes), len(used_subs), len(touched)
    nc = _get_program(ng, ns, nt)

    in_maps = []
    for c in range(N_CORES):
        rows = slice(c * BS, (c + 1) * BS)
        gene_t = gene_expr[rows, :].T[used_genes].astype(np.float16)  # [ng, BS]
        m = {
            "conc_t": np.ascontiguousarray(
                conc[rows, :].T[used_subs].astype(np.float16)
            ),
            **consts,
        }
        for i, g0 in enumerate(range(0, ng, 128)):
            m[f"gene_{i}"] = np.ascontiguousarray(gene_t[g0 : g0 + 128])
        in_maps.append(m)

    res = run_bass_kernel_spmd(nc, in_maps, core_ids=list(range(N_CORES)))

    # device ships the fp16 delta DT*d_conc on touched mets; finish
    # out = relu(conc + delta) in fp32 on the host (keeps conc's full
    # precision in the dominant term). Untouched mets keep delta == 0.
    out = conc.copy()
    for c in range(N_CORES):
        rows = slice(c * BS, (c + 1) * BS)
        delta = res.results[c]["out_t"].T.astype(np.float32)  # [BS, nt]
        out[rows, touched] += delta
    np.maximum(out, 0.0, out=out)
    return out
